# revision 15
# baseline (speedup 1.0000x reference)
"""DynamicGate MoE routing kernel for Trainium2 (8 NeuronCores, Bass/Tile).

Computes, for x[N,H], sim_matrix[H,E], gates[E]:
    logits = l2norm_rows(x) @ l2norm_cols(sim_matrix)
    thr    = sigmoid(gates)
    gated  = relu(logits - thr)
    mask   = (gated > 0), with top-1 fallback for all-inactive tokens
    probs  = softmax over active experts of gated
Returns (mask, probs, logits), all [N, E] fp32.

Sharding: data-parallel on the token dim across 8 cores (2048 tokens per
core); sim_matrix/gates replicated. No collectives needed.

Strategy (v2):
  - x is pre-rounded to FP32R (fp32 with 11 explicit mantissa bits) on the
    host - a bitwise no-op for DMA, and lets the PE run matmuls at 1
    cycle/row (4x the plain-fp32 rate) and transposes at 1.5 cycles/row.
  - logits are computed TRANSPOSED: for each 512-token tile,
    plg[64,512] += wn_c^T @ xt_c over 16 h-chunks, with the tiny wn as the
    stationary operand (64-column weight loads) and tokens as the wide
    moving operand.
  - per-token sum-of-squares runs as fused square+accumulate, split
    between ACT (activation Square accum_out) and DVE (stt accum_out).
  - PSUM->SBUF copies of transposed x are split between ACT and DVE.
  - epilogue: transpose logits^T back to [tok, E] blocks (fp32, exact),
    then mask/probs with argmax comparisons on full-fp32 values; bf16
    elementwise where precision allows; bf16 DMA-out, upcast on host.
"""

import sys

if "/opt/trn_rl_repo" not in sys.path:
    sys.path.insert(0, "/opt/trn_rl_repo")

import numpy as np

import concourse.bacc as bacc
import concourse.mybir as mybir
from concourse import bass_utils, masks
from concourse.tile import TileContext

F32 = mybir.dt.float32
F32R = mybir.dt.float32r
BF16 = mybir.dt.bfloat16
OP = mybir.AluOpType
AF = mybir.ActivationFunctionType
AX = mybir.AxisListType

N, H, E = 16384, 2048, 64
NCORES = 8
NLOC = N // NCORES     # 2048 tokens per core
PB = 128               # tokens per block (partition dim)
HC = H // 128          # 16 h-chunks
TB = 512               # tokens per tile
NBLK = TB // PB        # 4 blocks per tile
NTILE = NLOC // TB     # 4 tiles per core
EPS = 1e-12

# per-tile engine split knobs
SUMSQ_ON_DVE = 2       # of NBLK sumsq blocks, how many go to DVE (rest ACT)
COPIES_ON_DVE = 4      # of 8 xt copies per tile, how many go to DVE (rest ACT)


def build():
    nc = bacc.Bacc("TRN2", target_bir_lowering=False, debug=False)
    x_d = nc.dram_tensor("x", [NLOC, H], F32R, kind="ExternalInput")
    sim_d = nc.dram_tensor("sim", [H, E], F32, kind="ExternalInput")
    gates_d = nc.dram_tensor("gates", [1, E], F32, kind="ExternalInput")
    mask_d = nc.dram_tensor("mask", [NLOC, E], BF16, kind="ExternalOutput")
    probs_d = nc.dram_tensor("probs", [NLOC, E], BF16, kind="ExternalOutput")
    logits_d = nc.dram_tensor("logits", [NLOC, E], BF16, kind="ExternalOutput")

    with TileContext(nc) as tc:
        with (
            tc.tile_pool(name="const", bufs=1) as constp,
            tc.tile_pool(name="xin", bufs=10) as xinp,
            tc.tile_pool(name="xt", bufs=2) as xtp,
            tc.tile_pool(name="sq", bufs=1) as sqp,
            tc.tile_pool(name="ep", bufs=2) as epp,
            tc.tile_pool(name="sc", bufs=2) as scp,
            tc.tile_pool(name="psT", bufs=2, space="PSUM") as psT,
            tc.tile_pool(name="psL", bufs=2, space="PSUM") as psL,
            tc.tile_pool(name="psB", bufs=2, space="PSUM") as psB,
        ):
            # ---- constants -----------------------------------------------
            ident_f = constp.tile([128, 128], F32, name="ident_f")
            masks.make_identity(nc, ident_f)
            ident_r = constp.tile([128, 128], F32R, name="ident_r")
            nc.vector.tensor_copy(ident_r, ident_f)
            onesc = constp.tile([128, 1], F32, name="onesc")
            nc.gpsimd.memset(onesc, 1.0)
            onesr = constp.tile([1, 128], F32, name="onesr")
            nc.gpsimd.memset(onesr, 1.0)


            I32 = mybir.dt.int32
            MAGIC = 0x5F3759DF

            def emit_rsqrt(pool, src_ap, shape, tag, f_used=None):
                """rx = 1/sqrt(src) on DVE only: magic-constant + 2 Newton."""
                p, f = shape
                fu = f if f_used is None else f_used
                sa = src_ap[:, 0:fu]
                it = pool.tile([p, f], I32, name=tag + "_i",
                               tag=tag + "_i")[:, 0:fu]
                nc.vector.tensor_scalar(
                    out=it, in0=sa.bitcast(I32), scalar1=1, scalar2=None,
                    op0=OP.logical_shift_right,
                )
                nc.vector.tensor_scalar(
                    out=it, in0=it, scalar1=0xFFFFFFFF, scalar2=None,
                    op0=OP.bitwise_xor,
                )
                nc.vector.tensor_scalar(
                    out=it, in0=it, scalar1=MAGIC + 1, scalar2=None,
                    op0=OP.add,
                )
                y = it.bitcast(F32)
                t1 = pool.tile([p, f], F32, name=tag + "_t",
                               tag=tag + "_t")[:, 0:fu]
                for _ in range(2):
                    nc.vector.tensor_tensor(out=t1, in0=y, in1=y, op=OP.mult)
                    nc.vector.tensor_tensor(out=t1, in0=t1, in1=sa, op=OP.mult)
                    nc.vector.tensor_scalar(
                        out=t1, in0=t1, scalar1=-0.5, scalar2=1.5,
                        op0=OP.mult, op1=OP.add,
                    )
                    nc.vector.tensor_tensor(out=y, in0=y, in1=t1, op=OP.mult)
                return y

            wn = constp.tile([128, HC * E], F32, name="wn")
            g_row = constp.tile([1, E], F32, name="g_row")

            def emit_const_dmas():
                nc.sync.dma_start(
                    out=wn.rearrange("p (c e) -> p c e", e=E),
                    in_=sim_d.ap().rearrange("(c p) e -> p c e", p=128),
                )
                nc.sync.dma_start(out=g_row, in_=gates_d.ap())

            # wn_s: column-normalized sim, f32r, chunk-major [128, c, e]
            wn_s = constp.tile([128, HC, E], F32R, name="wn_s")
            thr_bb = constp.tile([128, E], BF16, name="thr_bb")

            def emit_wn_preamble():
                wnsq = constp.tile([128, HC * E], F32, name="wnsq")
                nc.scalar.square(wnsq, wn)
                csb = psB.tile([128, NBLK, E], F32, name="csb", tag="ptb")
                cs_ps = csb[0:1, 0, :]
                for c in range(HC):
                    nc.tensor.matmul(
                        cs_ps, lhsT=onesc, rhs=wnsq[:, c * E:(c + 1) * E],
                        start=(c == 0), stop=(c == HC - 1),
                    )
                # rwn = 1/max(sqrt(cs), EPS): DVE-only Newton rsqrt
                csm = constp.tile([1, E], F32, name="csm")
                nc.vector.tensor_scalar(
                    out=csm, in0=cs_ps, scalar1=EPS * EPS, scalar2=None,
                    op0=OP.max,
                )
                rwn = emit_rsqrt(constp, csm, (1, E), "rwn")

                # thr = sigmoid(g) = 1/(1+exp(-g))  (stays in the exp/ln set)
                eneg = constp.tile([1, E], F32, name="eneg")
                nc.scalar.activation(eneg, g_row, AF.Exp, scale=-1.0)
                nc.vector.tensor_scalar(
                    out=eneg, in0=eneg, scalar1=1.0, scalar2=None, op0=OP.add
                )
                thr_row = constp.tile([1, E], F32, name="thr_row")
                nc.vector.reciprocal(thr_row, eneg)

                # broadcast [1,E] rows to 128 partitions via rank-1 matmul
                bcb = psB.tile([128, NBLK, E], F32, name="bcb", tag="ptb")
                bc_ps = bcb.rearrange("p j e -> p (j e)")[:, 0:2 * E]
                nc.tensor.matmul(bc_ps[:, 0:E], lhsT=onesr, rhs=rwn,
                                 start=True, stop=True)
                nc.tensor.matmul(bc_ps[:, E:2 * E], lhsT=onesr, rhs=thr_row,
                                 start=True, stop=True)
                rwn_b = constp.tile([128, E], F32, name="rwn_b")
                nc.scalar.copy(rwn_b, bc_ps[:, 0:E])
                nc.scalar.copy(thr_bb, bc_ps[:, E:2 * E])

                # wn_s[p, c, e] = wn[p, c*E+e] * rwn_b[p, e]  (f32r rounded)
                nc.vector.tensor_tensor(
                    out=wn_s,
                    in0=wn.rearrange("p (c e) -> p c e", e=E),
                    in1=rwn_b.unsqueeze(1).broadcast_to([128, HC, E]),
                    op=OP.mult,
                )

            # ---- main loop: tapered tiles of token blocks ----------------
            TILES = [(0, 2), (2, 4), (6, 4), (10, 4), (14, 2)]
            x_tiles = {}
            dma_engines = [nc.sync, nc.scalar, nc.sync, nc.gpsimd]
            next_pf = [0]

            def prefetch():
                b = next_pf[0]
                if b >= NLOC // PB:
                    return
                next_pf[0] += 1
                t = xinp.tile([128, H], F32R, name="x_nat", tag="x_nat")
                dma_engines[b % 4].dma_start(
                    out=t, in_=x_d.ap()[b * PB:(b + 1) * PB, :]
                )
                x_tiles[b] = t

            for _ in range(4):
                prefetch()
            emit_const_dmas()

            def emit_epilogue(t0, tb, ptb, rx):
                # -- epilogue on [128, tb, E] ------------------------------
                def bce(ap):   # [128, tb] -> [128, tb, E] stride-0
                    return ap.unsqueeze(2).broadcast_to([128, tb, E])

                pts = ptb[:, 0:tb, :]
                lmax = scp.tile([128, NBLK], F32, name="lmax", tag="lmax")[:, 0:tb]
                nc.vector.tensor_reduce(
                    out=lmax, in_=pts, axis=AX.X, op=OP.max,
                )
                onehot = epp.tile([128, NBLK, E], BF16, name="onehot",
                                  tag="onehot")[:, 0:tb, :]
                nc.vector.tensor_tensor(
                    out=onehot, in0=pts, in1=bce(lmax), op=OP.is_equal,
                )
                logits_bf = epp.tile([128, NBLK, E], BF16, name="logits_bf",
                                     tag="logits_bf")[:, 0:tb, :]
                nc.vector.tensor_tensor(
                    out=logits_bf, in0=pts, in1=bce(rx), op=OP.mult,
                )
                gsub = epp.tile([128, NBLK, E], BF16, name="gsub",
                                tag="gsub")[:, 0:tb, :]
                nc.vector.tensor_tensor(
                    out=gsub, in0=logits_bf,
                    in1=thr_bb.unsqueeze(1).broadcast_to([128, tb, E]),
                    op=OP.subtract,
                )
                ind = epp.tile([128, NBLK, E], BF16, name="ind",
                               tag="ind")[:, 0:tb, :]
                nc.vector.tensor_scalar(
                    out=ind, in0=gsub, scalar1=0.0, scalar2=None, op0=OP.is_gt,
                )
                nact = scp.tile([128, NBLK], F32, name="nact", tag="nact")[:, 0:tb]
                nc.vector.tensor_reduce(
                    out=nact, in_=ind, axis=AX.X, op=OP.add,
                )
                inact = scp.tile([128, NBLK], F32, name="inact",
                                 tag="inact")[:, 0:tb]
                nc.vector.tensor_scalar(
                    out=inact, in0=nact, scalar1=0.0, scalar2=None,
                    op0=OP.is_equal,
                )
                maskt = epp.tile([128, NBLK, E], BF16, name="maskt",
                                 tag="maskt")[:, 0:tb, :]
                nc.vector.tensor_tensor(
                    out=maskt, in0=onehot, in1=bce(inact), op=OP.mult,
                )
                nc.vector.tensor_tensor(
                    out=maskt, in0=maskt, in1=ind, op=OP.add,
                )
                # probs = mask*exp(gsub) / sum(mask*exp(gsub))  (gmax-free:
                # gsub is small, and fallback rows renormalize to 1)
                ex = epp.tile([128, NBLK, E], BF16, name="ex",
                              tag="ex")[:, 0:tb, :]
                nc.scalar.activation(ex, gsub, AF.Exp)
                me = epp.tile([128, NBLK, E], BF16, name="me",
                              tag="me")[:, 0:tb, :]
                nc.vector.tensor_tensor(
                    out=me, in0=ex, in1=maskt, op=OP.mult,
                )
                sesum = scp.tile([128, NBLK], F32, name="sesum",
                                 tag="sesum")[:, 0:tb]
                nc.vector.tensor_reduce(
                    out=sesum, in_=me, axis=AX.X, op=OP.add,
                )
                rs = scp.tile([128, NBLK], F32, name="rs", tag="rs")[:, 0:tb]
                nc.vector.reciprocal(rs, sesum)
                probs = epp.tile([128, NBLK, E], BF16, name="probs",
                                 tag="probs")[:, 0:tb, :]
                nc.vector.tensor_tensor(
                    out=probs, in0=me, in1=bce(rs), op=OP.mult,
                )

                gtok = slice(t0 * PB, (t0 + tb) * PB)
                for out_d, osrc in ((mask_d, maskt), (probs_d, probs),
                                    (logits_d, logits_bf)):
                    nc.sync.dma_start(
                        out=out_d.ap()[gtok, :].rearrange(
                            "(j p) e -> p j e", p=128),
                        in_=osrc,
                    )

            pending = None
            for ti, (b0, tb) in enumerate(TILES):
                blocks = [x_tiles.pop(b0 + j) for j in range(tb)]
                for _ in range(4):
                    prefetch()
                tw = tb * PB   # tokens in this tile

                # -- transpose rounds with interleaved accumulation --------
                xt = xtp.tile([128, HC, TB], F32R, name="xt", tag="xt")
                plg = psL.tile([64, TB], F32, name="plg", tag="plg")
                for cp in range(HC // 2):
                    pt = psT.tile([128, 2, NBLK, 128], F32R, name="pt",
                                  tag="pt")
                    for k in range(2):
                        c = 2 * cp + k
                        for j in range(tb):
                            nc.tensor.transpose(
                                pt[:, k, j, :],
                                blocks[j][:, c * 128:(c + 1) * 128],
                                ident_r,
                            )
                    dst = xt[:, 2 * cp:2 * cp + 2, 0:tw]
                    src = pt[:, :, 0:tb, :].rearrange("p k j t -> p k (j t)")
                    if cp % 4 == 1:
                        nc.vector.tensor_copy(dst, src)
                    else:
                        nc.scalar.copy(dst, src)
                    if ti == 0:
                        continue
                    for k in range(2):
                        c = 2 * cp + k
                        nc.tensor.matmul(
                            plg[:, 0:tw], lhsT=wn_s[:, c, :],
                            rhs=xt[:, c, 0:tw],
                            start=(c == 0), stop=(c == HC - 1),
                        )
                if ti == 0:
                    # preamble PE work runs behind tile-0 transposes
                    emit_wn_preamble()
                    for c in range(HC):
                        nc.tensor.matmul(
                            plg[:, 0:tw], lhsT=wn_s[:, c, :],
                            rhs=xt[:, c, 0:tw],
                            start=(c == 0), stop=(c == HC - 1),
                        )

                # -- sumsq per block (emitted after copies: lower priority) -
                ssq = scp.tile([128, NBLK], F32, name="ssq", tag="ssq")
                for j in range(tb):
                    if j < SUMSQ_ON_DVE:
                        sq = sqp.tile([128, H], F32, name="sqd", tag="sqd")
                        nc.vector.scalar_tensor_tensor(
                            out=sq, in0=blocks[j], scalar=1.0, in1=blocks[j],
                            op0=OP.mult, op1=OP.mult,
                            accum_out=ssq[:, j:j + 1],
                        )
                    else:
                        sq = sqp.tile([128, H], F32, name="sqa", tag="sqa")
                        nc.scalar.activation(
                            sq, blocks[j], AF.Square,
                            accum_out=ssq[:, j:j + 1],
                        )
                # rx = 1/max(sqrt(ssq), eps): DVE-only Newton rsqrt
                ssqm = scp.tile([128, NBLK], F32, name="ssqm", tag="ssqm")
                nc.vector.tensor_scalar(
                    out=ssqm[:, 0:tb], in0=ssq[:, 0:tb], scalar1=EPS * EPS,
                    scalar2=None, op0=OP.max,
                )
                rx = emit_rsqrt(scp, ssqm, (128, NBLK), "rx", f_used=tb)

                # -- transpose back to [tok, E] blocks (full fp32) ---------
                lgT = epp.tile([64, TB], F32, name="lgT", tag="lgT")
                nc.scalar.copy(lgT[:, 0:tw], plg[:, 0:tw])
                ptb = psB.tile([128, NBLK, E], F32, name="ptb", tag="ptb")
                for j in range(tb):
                    nc.tensor.transpose(
                        ptb[:, j, :], lgT[:, j * 128:(j + 1) * 128],
                        ident_f[0:64, 0:64],
                    )

                # previous tile's epilogue drains while this tile streams
                if pending is not None:
                    emit_epilogue(*pending)
                pending = (b0, tb, ptb, rx)
            emit_epilogue(*pending)

    nc.compile()
    return nc


_NC_CACHE = {}


def _get_nc():
    if "nc" not in _NC_CACHE:
        _NC_CACHE["nc"] = build()
    return _NC_CACHE["nc"]


def _round_f32r(a):
    """Round fp32 to FP32R (11 explicit mantissa bits), nearest-even."""
    b = np.ascontiguousarray(a, dtype=np.float32).view(np.uint32)
    hi = b >> np.uint32(12)
    low = b & np.uint32(0xFFF)
    rnd = (low > np.uint32(0x800)) | (
        (low == np.uint32(0x800)) & ((hi & np.uint32(1)) == np.uint32(1))
    )
    out = (hi + rnd.astype(np.uint32)) << np.uint32(12)
    return out.view(np.float32)


def make_in_maps(x, sim_matrix, gates):
    x = _round_f32r(np.asarray(x, dtype=np.float32))
    sim = np.ascontiguousarray(np.asarray(sim_matrix, dtype=np.float32))
    g = np.ascontiguousarray(np.asarray(gates, dtype=np.float32)).reshape(1, E)
    return [
        {"x": x[c * NLOC:(c + 1) * NLOC], "sim": sim, "gates": g}
        for c in range(NCORES)
    ]


def kernel(x, sim_matrix, gates):
    nc = _get_nc()
    in_maps = make_in_maps(x, sim_matrix, gates)
    res = bass_utils.run_bass_kernel_spmd(nc, in_maps, core_ids=list(range(NCORES)))
    outs = []
    for name in ("mask", "probs", "logits"):
        outs.append(np.concatenate(
            [np.asarray(res.results[c][name], dtype=np.float32)
             for c in range(NCORES)], axis=0))
    return tuple(outs)


# revision 16
# speedup vs baseline: 1.1154x; 1.1154x over previous
"""DynamicGate MoE routing kernel for Trainium2 (8 NeuronCores, Bass/Tile).

Computes, for x[N,H], sim_matrix[H,E], gates[E]:
    logits = l2norm_rows(x) @ l2norm_cols(sim_matrix)
    thr    = sigmoid(gates)
    gated  = relu(logits - thr)
    mask   = (gated > 0), with top-1 fallback for all-inactive tokens
    probs  = softmax over active experts of gated
Returns (mask, probs, logits), all [N, E] fp32.

Sharding: data-parallel on the token dim across 8 cores (2048 tokens per
core); sim_matrix/gates replicated. No collectives needed.

Strategy (v2):
  - x is pre-rounded to FP32R (fp32 with 11 explicit mantissa bits) on the
    host - a bitwise no-op for DMA, and lets the PE run matmuls at 1
    cycle/row (4x the plain-fp32 rate) and transposes at 1.5 cycles/row.
  - logits are computed TRANSPOSED: for each 512-token tile,
    plg[64,512] += wn_c^T @ xt_c over 16 h-chunks, with the tiny wn as the
    stationary operand (64-column weight loads) and tokens as the wide
    moving operand.
  - per-token sum-of-squares runs as fused square+accumulate, split
    between ACT (activation Square accum_out) and DVE (stt accum_out).
  - PSUM->SBUF copies of transposed x are split between ACT and DVE.
  - epilogue: transpose logits^T back to [tok, E] blocks (fp32, exact),
    then mask/probs with argmax comparisons on full-fp32 values; bf16
    elementwise where precision allows; bf16 DMA-out, upcast on host.
"""

import sys

if "/opt/trn_rl_repo" not in sys.path:
    sys.path.insert(0, "/opt/trn_rl_repo")

import numpy as np

import concourse.bacc as bacc
import concourse.mybir as mybir
from concourse import bass_utils, masks
from concourse.tile import TileContext

F32 = mybir.dt.float32
F32R = mybir.dt.float32r
BF16 = mybir.dt.bfloat16
OP = mybir.AluOpType
AF = mybir.ActivationFunctionType
AX = mybir.AxisListType

N, H, E = 16384, 2048, 64
NCORES = 8
NLOC = N // NCORES     # 2048 tokens per core
PB = 128               # tokens per block (partition dim)
HC = H // 128          # 16 h-chunks
TB = 512               # tokens per tile
NBLK = TB // PB        # 4 blocks per tile
NTILE = NLOC // TB     # 4 tiles per core
EPS = 1e-12

# per-tile engine split knobs
SUMSQ_ON_DVE = 2       # of NBLK sumsq blocks, how many go to DVE (rest ACT)
COPIES_ON_DVE = 4      # of 8 xt copies per tile, how many go to DVE (rest ACT)


def build():
    nc = bacc.Bacc("TRN2", target_bir_lowering=False, debug=False)
    x_d = nc.dram_tensor("x", [NLOC, H], F32R, kind="ExternalInput")
    sim_d = nc.dram_tensor("sim", [H, E], F32, kind="ExternalInput")
    gates_d = nc.dram_tensor("gates", [1, E], F32, kind="ExternalInput")
    mask_d = nc.dram_tensor("mask", [NLOC, E], BF16, kind="ExternalOutput")
    probs_d = nc.dram_tensor("probs", [NLOC, E], BF16, kind="ExternalOutput")
    logits_d = nc.dram_tensor("logits", [NLOC, E], BF16, kind="ExternalOutput")

    with TileContext(nc) as tc:
        with (
            tc.tile_pool(name="const", bufs=1) as constp,
            tc.tile_pool(name="xin", bufs=10) as xinp,
            tc.tile_pool(name="xt", bufs=2) as xtp,
            tc.tile_pool(name="sq", bufs=1) as sqp,
            tc.tile_pool(name="ep", bufs=2) as epp,
            tc.tile_pool(name="sc", bufs=2) as scp,
            tc.tile_pool(name="psT", bufs=2, space="PSUM") as psT,
            tc.tile_pool(name="psL", bufs=2, space="PSUM") as psL,
            tc.tile_pool(name="psB", bufs=2, space="PSUM") as psB,
        ):
            # ---- constants -----------------------------------------------
            ident_f = constp.tile([128, 128], F32, name="ident_f")
            masks.make_identity(nc, ident_f)
            ident_r = constp.tile([128, 128], F32R, name="ident_r")
            nc.vector.tensor_copy(ident_r, ident_f)
            onesc = constp.tile([128, 1], F32, name="onesc")
            nc.gpsimd.memset(onesc, 1.0)
            onesr = constp.tile([1, 128], F32, name="onesr")
            nc.gpsimd.memset(onesr, 1.0)


            I32 = mybir.dt.int32
            MAGIC = 0x5F3759DF

            def emit_rsqrt(pool, src_ap, shape, tag, f_used=None):
                """rx = 1/sqrt(src) on DVE only: magic-constant + 2 Newton."""
                p, f = shape
                fu = f if f_used is None else f_used
                sa = src_ap[:, 0:fu]
                it = pool.tile([p, f], I32, name=tag + "_i",
                               tag=tag + "_i")[:, 0:fu]
                nc.vector.tensor_scalar(
                    out=it, in0=sa.bitcast(I32), scalar1=1, scalar2=None,
                    op0=OP.logical_shift_right,
                )
                nc.vector.tensor_scalar(
                    out=it, in0=it, scalar1=0xFFFFFFFF, scalar2=None,
                    op0=OP.bitwise_xor,
                )
                nc.vector.tensor_scalar(
                    out=it, in0=it, scalar1=MAGIC + 1, scalar2=None,
                    op0=OP.add,
                )
                y = it.bitcast(F32)
                t1 = pool.tile([p, f], F32, name=tag + "_t",
                               tag=tag + "_t")[:, 0:fu]
                for _ in range(2):
                    nc.vector.tensor_tensor(out=t1, in0=y, in1=y, op=OP.mult)
                    nc.vector.tensor_tensor(out=t1, in0=t1, in1=sa, op=OP.mult)
                    nc.vector.tensor_scalar(
                        out=t1, in0=t1, scalar1=-0.5, scalar2=1.5,
                        op0=OP.mult, op1=OP.add,
                    )
                    nc.vector.tensor_tensor(out=y, in0=y, in1=t1, op=OP.mult)
                return y

            wn = constp.tile([128, HC * E], F32, name="wn")
            g_row = constp.tile([1, E], F32, name="g_row")

            def emit_const_dmas():
                nc.sync.dma_start(
                    out=wn.rearrange("p (c e) -> p c e", e=E),
                    in_=sim_d.ap().rearrange("(c p) e -> p c e", p=128),
                )
                nc.sync.dma_start(out=g_row, in_=gates_d.ap())

            # wn_s: column-normalized sim, f32r, chunk-major [128, c, e]
            wn_s = constp.tile([128, HC, E], F32R, name="wn_s")
            thr_bb = constp.tile([128, E], BF16, name="thr_bb")

            def emit_wn_preamble():
                wnsq = constp.tile([128, HC * E], F32, name="wnsq")
                nc.scalar.square(wnsq, wn)
                csb = psB.tile([128, NBLK, E], F32, name="csb", tag="ptb")
                cs_ps = csb[0:1, 0, :]
                for c in range(HC):
                    nc.tensor.matmul(
                        cs_ps, lhsT=onesc, rhs=wnsq[:, c * E:(c + 1) * E],
                        start=(c == 0), stop=(c == HC - 1),
                    )
                # rwn = 1/max(sqrt(cs), EPS): DVE-only Newton rsqrt
                csm = constp.tile([1, E], F32, name="csm")
                nc.vector.tensor_scalar(
                    out=csm, in0=cs_ps, scalar1=EPS * EPS, scalar2=None,
                    op0=OP.max,
                )
                rwn = emit_rsqrt(constp, csm, (1, E), "rwn")

                # thr = sigmoid(g) = 1/(1+exp(-g))  (stays in the exp/ln set)
                eneg = constp.tile([1, E], F32, name="eneg")
                nc.scalar.activation(eneg, g_row, AF.Exp, scale=-1.0)
                nc.vector.tensor_scalar(
                    out=eneg, in0=eneg, scalar1=1.0, scalar2=None, op0=OP.add
                )
                thr_row = constp.tile([1, E], F32, name="thr_row")
                nc.vector.reciprocal(thr_row, eneg)

                # broadcast [1,E] rows to 128 partitions via rank-1 matmul
                bcb = psB.tile([128, NBLK, E], F32, name="bcb", tag="ptb")
                bc_ps = bcb.rearrange("p j e -> p (j e)")[:, 0:2 * E]
                nc.tensor.matmul(bc_ps[:, 0:E], lhsT=onesr, rhs=rwn,
                                 start=True, stop=True)
                nc.tensor.matmul(bc_ps[:, E:2 * E], lhsT=onesr, rhs=thr_row,
                                 start=True, stop=True)
                rwn_b = constp.tile([128, E], F32, name="rwn_b")
                nc.scalar.copy(rwn_b, bc_ps[:, 0:E])
                nc.scalar.copy(thr_bb, bc_ps[:, E:2 * E])

                # wn_s[p, c, e] = wn[p, c*E+e] * rwn_b[p, e]  (f32r rounded)
                nc.vector.tensor_tensor(
                    out=wn_s,
                    in0=wn.rearrange("p (c e) -> p c e", e=E),
                    in1=rwn_b.unsqueeze(1).broadcast_to([128, HC, E]),
                    op=OP.mult,
                )

            # ---- main loop: tapered tiles of token blocks ----------------
            TILES = [(0, 2), (2, 4), (6, 4), (10, 4), (14, 2)]
            x_tiles = {}
            dma_engines = [nc.sync, nc.sync, nc.sync, nc.sync]
            next_pf = [0]

            def prefetch():
                b = next_pf[0]
                if b >= NLOC // PB:
                    return
                next_pf[0] += 1
                t = xinp.tile([128, H], F32R, name="x_nat", tag="x_nat")
                dma_engines[b % 4].dma_start(
                    out=t, in_=x_d.ap()[b * PB:(b + 1) * PB, :]
                )
                x_tiles[b] = t

            for _ in range(4):
                prefetch()
            emit_const_dmas()

            def emit_epilogue(t0, tb, ptb, rx):
                # -- epilogue on [128, tb, E] ------------------------------
                def bce(ap):   # [128, tb] -> [128, tb, E] stride-0
                    return ap.unsqueeze(2).broadcast_to([128, tb, E])

                pts = ptb[:, 0:tb, :]
                lmax = scp.tile([128, NBLK], F32, name="lmax", tag="lmax")[:, 0:tb]
                nc.vector.tensor_reduce(
                    out=lmax, in_=pts, axis=AX.X, op=OP.max,
                )
                onehot = epp.tile([128, NBLK, E], BF16, name="onehot",
                                  tag="onehot")[:, 0:tb, :]
                nc.vector.tensor_tensor(
                    out=onehot, in0=pts, in1=bce(lmax), op=OP.is_equal,
                )
                logits_bf = epp.tile([128, NBLK, E], BF16, name="logits_bf",
                                     tag="logits_bf")[:, 0:tb, :]
                nc.vector.tensor_tensor(
                    out=logits_bf, in0=pts, in1=bce(rx), op=OP.mult,
                )
                gsub = epp.tile([128, NBLK, E], BF16, name="gsub",
                                tag="gsub")[:, 0:tb, :]
                nc.vector.tensor_tensor(
                    out=gsub, in0=logits_bf,
                    in1=thr_bb.unsqueeze(1).broadcast_to([128, tb, E]),
                    op=OP.subtract,
                )
                ind = epp.tile([128, NBLK, E], BF16, name="ind",
                               tag="ind")[:, 0:tb, :]
                nc.vector.tensor_scalar(
                    out=ind, in0=gsub, scalar1=0.0, scalar2=None, op0=OP.is_gt,
                )
                nact = scp.tile([128, NBLK], F32, name="nact", tag="nact")[:, 0:tb]
                nc.vector.tensor_reduce(
                    out=nact, in_=ind, axis=AX.X, op=OP.add,
                )
                inact = scp.tile([128, NBLK], F32, name="inact",
                                 tag="inact")[:, 0:tb]
                nc.vector.tensor_scalar(
                    out=inact, in0=nact, scalar1=0.0, scalar2=None,
                    op0=OP.is_equal,
                )
                maskt = epp.tile([128, NBLK, E], BF16, name="maskt",
                                 tag="maskt")[:, 0:tb, :]
                nc.vector.tensor_tensor(
                    out=maskt, in0=onehot, in1=bce(inact), op=OP.mult,
                )
                nc.vector.tensor_tensor(
                    out=maskt, in0=maskt, in1=ind, op=OP.add,
                )
                # probs = mask*exp(gsub) / sum(mask*exp(gsub))  (gmax-free:
                # gsub is small, and fallback rows renormalize to 1)
                ex = epp.tile([128, NBLK, E], BF16, name="ex",
                              tag="ex")[:, 0:tb, :]
                nc.scalar.activation(ex, gsub, AF.Exp)
                me = epp.tile([128, NBLK, E], BF16, name="me",
                              tag="me")[:, 0:tb, :]
                nc.vector.tensor_tensor(
                    out=me, in0=ex, in1=maskt, op=OP.mult,
                )
                sesum = scp.tile([128, NBLK], F32, name="sesum",
                                 tag="sesum")[:, 0:tb]
                nc.vector.tensor_reduce(
                    out=sesum, in_=me, axis=AX.X, op=OP.add,
                )
                rs = scp.tile([128, NBLK], F32, name="rs", tag="rs")[:, 0:tb]
                nc.vector.reciprocal(rs, sesum)
                probs = epp.tile([128, NBLK, E], BF16, name="probs",
                                 tag="probs")[:, 0:tb, :]
                nc.vector.tensor_tensor(
                    out=probs, in0=me, in1=bce(rs), op=OP.mult,
                )

                gtok = slice(t0 * PB, (t0 + tb) * PB)
                for out_d, osrc in ((mask_d, maskt), (probs_d, probs),
                                    (logits_d, logits_bf)):
                    nc.sync.dma_start(
                        out=out_d.ap()[gtok, :].rearrange(
                            "(j p) e -> p j e", p=128),
                        in_=osrc,
                    )

            pending = None
            for ti, (b0, tb) in enumerate(TILES):
                blocks = [x_tiles.pop(b0 + j) for j in range(tb)]
                for _ in range(4):
                    prefetch()
                tw = tb * PB   # tokens in this tile

                # -- transpose rounds with interleaved accumulation --------
                xt = xtp.tile([128, HC, TB], F32R, name="xt", tag="xt")
                plg = psL.tile([64, TB], F32, name="plg", tag="plg")
                for cp in range(HC // 2):
                    pt = psT.tile([128, 2, NBLK, 128], F32R, name="pt",
                                  tag="pt")
                    for k in range(2):
                        c = 2 * cp + k
                        for j in range(tb):
                            nc.tensor.transpose(
                                pt[:, k, j, :],
                                blocks[j][:, c * 128:(c + 1) * 128],
                                ident_r,
                            )
                    dst = xt[:, 2 * cp:2 * cp + 2, 0:tw]
                    src = pt[:, :, 0:tb, :].rearrange("p k j t -> p k (j t)")
                    if cp % 4 == 1:
                        nc.vector.tensor_copy(dst, src)
                    else:
                        nc.scalar.copy(dst, src)
                    if ti == 0:
                        continue
                    for k in range(2):
                        c = 2 * cp + k
                        nc.tensor.matmul(
                            plg[:, 0:tw], lhsT=wn_s[:, c, :],
                            rhs=xt[:, c, 0:tw],
                            start=(c == 0), stop=(c == HC - 1),
                        )
                if ti == 0:
                    # preamble PE work runs behind tile-0 transposes
                    emit_wn_preamble()
                    for c in range(HC):
                        nc.tensor.matmul(
                            plg[:, 0:tw], lhsT=wn_s[:, c, :],
                            rhs=xt[:, c, 0:tw],
                            start=(c == 0), stop=(c == HC - 1),
                        )

                # -- sumsq per block (emitted after copies: lower priority) -
                ssq = scp.tile([128, NBLK], F32, name="ssq", tag="ssq")
                for j in range(tb):
                    if j < SUMSQ_ON_DVE:
                        sq = sqp.tile([128, H], F32, name="sqd", tag="sqd")
                        nc.vector.scalar_tensor_tensor(
                            out=sq, in0=blocks[j], scalar=1.0, in1=blocks[j],
                            op0=OP.mult, op1=OP.mult,
                            accum_out=ssq[:, j:j + 1],
                        )
                    else:
                        sq = sqp.tile([128, H], F32, name="sqa", tag="sqa")
                        nc.scalar.activation(
                            sq, blocks[j], AF.Square,
                            accum_out=ssq[:, j:j + 1],
                        )
                # rx = 1/max(sqrt(ssq), eps): DVE-only Newton rsqrt
                ssqm = scp.tile([128, NBLK], F32, name="ssqm", tag="ssqm")
                nc.vector.tensor_scalar(
                    out=ssqm[:, 0:tb], in0=ssq[:, 0:tb], scalar1=EPS * EPS,
                    scalar2=None, op0=OP.max,
                )
                rx = emit_rsqrt(scp, ssqm, (128, NBLK), "rx", f_used=tb)

                # -- transpose back to [tok, E] blocks (full fp32) ---------
                lgT = epp.tile([64, TB], F32, name="lgT", tag="lgT")
                nc.scalar.copy(lgT[:, 0:tw], plg[:, 0:tw])
                ptb = psB.tile([128, NBLK, E], F32, name="ptb", tag="ptb")
                for j in range(tb):
                    nc.tensor.transpose(
                        ptb[:, j, :], lgT[:, j * 128:(j + 1) * 128],
                        ident_f[0:64, 0:64],
                    )

                # previous tile's epilogue drains while this tile streams
                if pending is not None:
                    emit_epilogue(*pending)
                pending = (b0, tb, ptb, rx)
            emit_epilogue(*pending)

    nc.compile()
    return nc


_NC_CACHE = {}


def _get_nc():
    if "nc" not in _NC_CACHE:
        _NC_CACHE["nc"] = build()
    return _NC_CACHE["nc"]


def _round_f32r(a):
    """Round fp32 to FP32R (11 explicit mantissa bits), nearest-even."""
    b = np.ascontiguousarray(a, dtype=np.float32).view(np.uint32)
    hi = b >> np.uint32(12)
    low = b & np.uint32(0xFFF)
    rnd = (low > np.uint32(0x800)) | (
        (low == np.uint32(0x800)) & ((hi & np.uint32(1)) == np.uint32(1))
    )
    out = (hi + rnd.astype(np.uint32)) << np.uint32(12)
    return out.view(np.float32)


def make_in_maps(x, sim_matrix, gates):
    x = _round_f32r(np.asarray(x, dtype=np.float32))
    sim = np.ascontiguousarray(np.asarray(sim_matrix, dtype=np.float32))
    g = np.ascontiguousarray(np.asarray(gates, dtype=np.float32)).reshape(1, E)
    return [
        {"x": x[c * NLOC:(c + 1) * NLOC], "sim": sim, "gates": g}
        for c in range(NCORES)
    ]


def kernel(x, sim_matrix, gates):
    nc = _get_nc()
    in_maps = make_in_maps(x, sim_matrix, gates)
    res = bass_utils.run_bass_kernel_spmd(nc, in_maps, core_ids=list(range(NCORES)))
    outs = []
    for name in ("mask", "probs", "logits"):
        outs.append(np.concatenate(
            [np.asarray(res.results[c][name], dtype=np.float32)
             for c in range(NCORES)], axis=0))
    return tuple(outs)


# revision 17
# speedup vs baseline: 1.1722x; 1.0509x over previous
"""DynamicGate MoE routing kernel for Trainium2 (8 NeuronCores, Bass/Tile).

Computes, for x[N,H], sim_matrix[H,E], gates[E]:
    logits = l2norm_rows(x) @ l2norm_cols(sim_matrix)
    thr    = sigmoid(gates)
    gated  = relu(logits - thr)
    mask   = (gated > 0), with top-1 fallback for all-inactive tokens
    probs  = softmax over active experts of gated
Returns (mask, probs, logits), all [N, E] fp32.

Sharding: data-parallel on the token dim across 8 cores (2048 tokens per
core); sim_matrix/gates replicated. No collectives needed.

Strategy (v2):
  - x is pre-rounded to FP32R (fp32 with 11 explicit mantissa bits) on the
    host - a bitwise no-op for DMA, and lets the PE run matmuls at 1
    cycle/row (4x the plain-fp32 rate) and transposes at 1.5 cycles/row.
  - logits are computed TRANSPOSED: for each 512-token tile,
    plg[64,512] += wn_c^T @ xt_c over 16 h-chunks, with the tiny wn as the
    stationary operand (64-column weight loads) and tokens as the wide
    moving operand.
  - per-token sum-of-squares runs as fused square+accumulate, split
    between ACT (activation Square accum_out) and DVE (stt accum_out).
  - PSUM->SBUF copies of transposed x are split between ACT and DVE.
  - epilogue: transpose logits^T back to [tok, E] blocks (fp32, exact),
    then mask/probs with argmax comparisons on full-fp32 values; bf16
    elementwise where precision allows; bf16 DMA-out, upcast on host.
"""

import sys

if "/opt/trn_rl_repo" not in sys.path:
    sys.path.insert(0, "/opt/trn_rl_repo")

import numpy as np

import concourse.bacc as bacc
import concourse.mybir as mybir
from concourse import bass_utils, masks
from concourse.tile import TileContext

F32 = mybir.dt.float32
F32R = mybir.dt.float32r
BF16 = mybir.dt.bfloat16
OP = mybir.AluOpType
AF = mybir.ActivationFunctionType
AX = mybir.AxisListType

N, H, E = 16384, 2048, 64
NCORES = 8
NLOC = N // NCORES     # 2048 tokens per core
PB = 128               # tokens per block (partition dim)
HC = H // 128          # 16 h-chunks
TB = 512               # tokens per tile
NBLK = TB // PB        # 4 blocks per tile
NTILE = NLOC // TB     # 4 tiles per core
EPS = 1e-12

# per-tile engine split knobs
SUMSQ_ON_DVE = 2       # of NBLK sumsq blocks, how many go to DVE (rest ACT)
COPIES_ON_DVE = 4      # of 8 xt copies per tile, how many go to DVE (rest ACT)


def build():
    nc = bacc.Bacc("TRN2", target_bir_lowering=False, debug=False)
    x_d = nc.dram_tensor("x", [NLOC, H], F32R, kind="ExternalInput")
    sim_d = nc.dram_tensor("sim", [H, E], F32, kind="ExternalInput")
    gates_d = nc.dram_tensor("gates", [1, E], F32, kind="ExternalInput")
    mask_d = nc.dram_tensor("mask", [NLOC, E], BF16, kind="ExternalOutput")
    probs_d = nc.dram_tensor("probs", [NLOC, E], BF16, kind="ExternalOutput")
    logits_d = nc.dram_tensor("logits", [NLOC, E], BF16, kind="ExternalOutput")

    with TileContext(nc) as tc:
        with (
            tc.tile_pool(name="const", bufs=1) as constp,
            tc.tile_pool(name="xin", bufs=10) as xinp,
            tc.tile_pool(name="xt", bufs=2) as xtp,
            tc.tile_pool(name="sq", bufs=2) as sqp,
            tc.tile_pool(name="ep", bufs=2) as epp,
            tc.tile_pool(name="sc", bufs=2) as scp,
            tc.tile_pool(name="psT", bufs=3, space="PSUM") as psT,
            tc.tile_pool(name="psL", bufs=1, space="PSUM") as psL,
            tc.tile_pool(name="psB", bufs=1, space="PSUM") as psB,
        ):
            # ---- constants -----------------------------------------------
            ident_f = constp.tile([128, 128], F32, name="ident_f")
            masks.make_identity(nc, ident_f)
            ident_r = constp.tile([128, 128], F32R, name="ident_r")
            nc.vector.tensor_copy(ident_r, ident_f)
            onesc = constp.tile([128, 1], F32, name="onesc")
            nc.gpsimd.memset(onesc, 1.0)
            onesr = constp.tile([1, 128], F32, name="onesr")
            nc.gpsimd.memset(onesr, 1.0)


            I32 = mybir.dt.int32
            MAGIC = 0x5F3759DF

            def emit_rsqrt(pool, src_ap, shape, tag, f_used=None):
                """rx = 1/sqrt(src) on DVE only: magic-constant + 2 Newton."""
                p, f = shape
                fu = f if f_used is None else f_used
                sa = src_ap[:, 0:fu]
                it = pool.tile([p, f], I32, name=tag + "_i",
                               tag=tag + "_i")[:, 0:fu]
                nc.vector.tensor_scalar(
                    out=it, in0=sa.bitcast(I32), scalar1=1, scalar2=None,
                    op0=OP.logical_shift_right,
                )
                nc.vector.tensor_scalar(
                    out=it, in0=it, scalar1=0xFFFFFFFF, scalar2=None,
                    op0=OP.bitwise_xor,
                )
                nc.vector.tensor_scalar(
                    out=it, in0=it, scalar1=MAGIC + 1, scalar2=None,
                    op0=OP.add,
                )
                y = it.bitcast(F32)
                t1 = pool.tile([p, f], F32, name=tag + "_t",
                               tag=tag + "_t")[:, 0:fu]
                for _ in range(2):
                    nc.vector.tensor_tensor(out=t1, in0=y, in1=y, op=OP.mult)
                    nc.vector.tensor_tensor(out=t1, in0=t1, in1=sa, op=OP.mult)
                    nc.vector.tensor_scalar(
                        out=t1, in0=t1, scalar1=-0.5, scalar2=1.5,
                        op0=OP.mult, op1=OP.add,
                    )
                    nc.vector.tensor_tensor(out=y, in0=y, in1=t1, op=OP.mult)
                return y

            wn = constp.tile([128, HC * E], F32, name="wn")
            g_row = constp.tile([1, E], F32, name="g_row")

            def emit_const_dmas():
                nc.sync.dma_start(
                    out=wn.rearrange("p (c e) -> p c e", e=E),
                    in_=sim_d.ap().rearrange("(c p) e -> p c e", p=128),
                )
                nc.sync.dma_start(out=g_row, in_=gates_d.ap())

            # wn_s: column-normalized sim, f32r, chunk-major [128, c, e]
            wn_s = constp.tile([128, HC, E], F32R, name="wn_s")
            thr_bb = constp.tile([128, E], BF16, name="thr_bb")

            def emit_wn_preamble():
                wnsq = constp.tile([128, HC * E], F32, name="wnsq")
                nc.scalar.square(wnsq, wn)
                csb = psB.tile([128, NBLK, E], F32, name="csb", tag="ptb")
                cs_ps = csb[0:1, 0, :]
                for c in range(HC):
                    nc.tensor.matmul(
                        cs_ps, lhsT=onesc, rhs=wnsq[:, c * E:(c + 1) * E],
                        start=(c == 0), stop=(c == HC - 1),
                    )
                # rwn = 1/max(sqrt(cs), EPS): DVE-only Newton rsqrt
                csm = constp.tile([1, E], F32, name="csm")
                nc.vector.tensor_scalar(
                    out=csm, in0=cs_ps, scalar1=EPS * EPS, scalar2=None,
                    op0=OP.max,
                )
                rwn = emit_rsqrt(constp, csm, (1, E), "rwn")

                # thr = sigmoid(g) = 1/(1+exp(-g))  (stays in the exp/ln set)
                eneg = constp.tile([1, E], F32, name="eneg")
                nc.scalar.activation(eneg, g_row, AF.Exp, scale=-1.0)
                nc.vector.tensor_scalar(
                    out=eneg, in0=eneg, scalar1=1.0, scalar2=None, op0=OP.add
                )
                thr_row = constp.tile([1, E], F32, name="thr_row")
                nc.vector.reciprocal(thr_row, eneg)

                # broadcast [1,E] rows to 128 partitions via rank-1 matmul
                bcb = psB.tile([128, NBLK, E], F32, name="bcb", tag="ptb")
                bc_ps = bcb.rearrange("p j e -> p (j e)")[:, 0:2 * E]
                nc.tensor.matmul(bc_ps[:, 0:E], lhsT=onesr, rhs=rwn,
                                 start=True, stop=True)
                nc.tensor.matmul(bc_ps[:, E:2 * E], lhsT=onesr, rhs=thr_row,
                                 start=True, stop=True)
                rwn_b = constp.tile([128, E], F32, name="rwn_b")
                nc.scalar.copy(rwn_b, bc_ps[:, 0:E])
                nc.scalar.copy(thr_bb, bc_ps[:, E:2 * E])

                # wn_s[p, c, e] = wn[p, c*E+e] * rwn_b[p, e]  (f32r rounded)
                nc.vector.tensor_tensor(
                    out=wn_s,
                    in0=wn.rearrange("p (c e) -> p c e", e=E),
                    in1=rwn_b.unsqueeze(1).broadcast_to([128, HC, E]),
                    op=OP.mult,
                )

            # ---- main loop: tapered tiles of token blocks ----------------
            TILES = [(0, 2), (2, 4), (6, 4), (10, 4), (14, 2)]
            x_tiles = {}
            dma_engines = [nc.sync, nc.sync, nc.sync, nc.sync]
            next_pf = [0]

            def prefetch():
                b = next_pf[0]
                if b >= NLOC // PB:
                    return
                next_pf[0] += 1
                t = xinp.tile([128, H], F32R, name="x_nat", tag="x_nat")
                dma_engines[b % 4].dma_start(
                    out=t, in_=x_d.ap()[b * PB:(b + 1) * PB, :]
                )
                x_tiles[b] = t

            for _ in range(4):
                prefetch()
            emit_const_dmas()

            def emit_epilogue(t0, tb, ptb, rx):
                # -- epilogue on [128, tb, E] ------------------------------
                def bce(ap):   # [128, tb] -> [128, tb, E] stride-0
                    return ap.unsqueeze(2).broadcast_to([128, tb, E])

                pts = ptb[:, 0:tb, :]
                lmax = scp.tile([128, NBLK], F32, name="lmax", tag="lmax")[:, 0:tb]
                nc.vector.tensor_reduce(
                    out=lmax, in_=pts, axis=AX.X, op=OP.max,
                )
                onehot = epp.tile([128, NBLK, E], BF16, name="onehot",
                                  tag="onehot")[:, 0:tb, :]
                nc.vector.tensor_tensor(
                    out=onehot, in0=pts, in1=bce(lmax), op=OP.is_equal,
                )
                logits_bf = epp.tile([128, NBLK, E], BF16, name="logits_bf",
                                     tag="logits_bf")[:, 0:tb, :]
                nc.vector.tensor_tensor(
                    out=logits_bf, in0=pts, in1=bce(rx), op=OP.mult,
                )
                gsub = epp.tile([128, NBLK, E], BF16, name="gsub",
                                tag="gsub")[:, 0:tb, :]
                nc.vector.tensor_tensor(
                    out=gsub, in0=logits_bf,
                    in1=thr_bb.unsqueeze(1).broadcast_to([128, tb, E]),
                    op=OP.subtract,
                )
                ind = epp.tile([128, NBLK, E], BF16, name="ind",
                               tag="ind")[:, 0:tb, :]
                nc.vector.tensor_scalar(
                    out=ind, in0=gsub, scalar1=0.0, scalar2=None, op0=OP.is_gt,
                )
                nact = scp.tile([128, NBLK], F32, name="nact", tag="nact")[:, 0:tb]
                nc.vector.tensor_reduce(
                    out=nact, in_=ind, axis=AX.X, op=OP.add,
                )
                inact = scp.tile([128, NBLK], F32, name="inact",
                                 tag="inact")[:, 0:tb]
                nc.vector.tensor_scalar(
                    out=inact, in0=nact, scalar1=0.0, scalar2=None,
                    op0=OP.is_equal,
                )
                maskt = epp.tile([128, NBLK, E], BF16, name="maskt",
                                 tag="maskt")[:, 0:tb, :]
                nc.vector.tensor_tensor(
                    out=maskt, in0=onehot, in1=bce(inact), op=OP.mult,
                )
                nc.vector.tensor_tensor(
                    out=maskt, in0=maskt, in1=ind, op=OP.add,
                )
                # probs = mask*exp(gsub) / sum(mask*exp(gsub))  (gmax-free:
                # gsub is small, and fallback rows renormalize to 1)
                ex = epp.tile([128, NBLK, E], BF16, name="ex",
                              tag="ex")[:, 0:tb, :]
                nc.scalar.activation(ex, gsub, AF.Exp)
                me = epp.tile([128, NBLK, E], BF16, name="me",
                              tag="me")[:, 0:tb, :]
                nc.vector.tensor_tensor(
                    out=me, in0=ex, in1=maskt, op=OP.mult,
                )
                sesum = scp.tile([128, NBLK], F32, name="sesum",
                                 tag="sesum")[:, 0:tb]
                nc.vector.tensor_reduce(
                    out=sesum, in_=me, axis=AX.X, op=OP.add,
                )
                rs = scp.tile([128, NBLK], F32, name="rs", tag="rs")[:, 0:tb]
                nc.vector.reciprocal(rs, sesum)
                probs = epp.tile([128, NBLK, E], BF16, name="probs",
                                 tag="probs")[:, 0:tb, :]
                nc.vector.tensor_tensor(
                    out=probs, in0=me, in1=bce(rs), op=OP.mult,
                )

                gtok = slice(t0 * PB, (t0 + tb) * PB)
                for out_d, osrc in ((mask_d, maskt), (probs_d, probs),
                                    (logits_d, logits_bf)):
                    nc.sync.dma_start(
                        out=out_d.ap()[gtok, :].rearrange(
                            "(j p) e -> p j e", p=128),
                        in_=osrc,
                    )

            pending = None
            for ti, (b0, tb) in enumerate(TILES):
                blocks = [x_tiles.pop(b0 + j) for j in range(tb)]
                for _ in range(4):
                    prefetch()
                tw = tb * PB   # tokens in this tile

                # -- transpose rounds with interleaved accumulation --------
                # sumsq runs as 512-wide square+accum pieces slotted between
                # the PSUM->SBUF copies so no engine sits on a 2us slab.
                xt = xtp.tile([128, HC, TB], F32R, name="xt", tag="xt")
                plg = psL.tile([64, TB], F32, name="plg", tag="plg")
                ssq_p = scp.tile([128, NBLK, 4], F32, name="ssq_p", tag="ssq_p")
                pieces = [(j, q) for j in range(tb) for q in range(4)]
                for cp in range(HC // 2):
                    pt = psT.tile([128, 2, NBLK, 128], F32R, name="pt",
                                  tag="pt")
                    for k in range(2):
                        c = 2 * cp + k
                        for j in range(tb):
                            nc.tensor.transpose(
                                pt[:, k, j, :],
                                blocks[j][:, c * 128:(c + 1) * 128],
                                ident_r,
                            )
                    dst = xt[:, 2 * cp:2 * cp + 2, 0:tw]
                    src = pt[:, :, 0:tb, :].rearrange("p k j t -> p k (j t)")
                    if cp % 4 == 1:
                        nc.vector.tensor_copy(dst, src)
                    else:
                        nc.scalar.copy(dst, src)
                    # two sumsq pieces per round
                    for _ in range(2):
                        if not pieces:
                            continue
                        j, q = pieces.pop(0)
                        xpc = blocks[j][:, q * 512:(q + 1) * 512]
                        if (j + q) % 2 == 0:
                            sq = sqp.tile([128, 512], F32, name="sqd", tag="sqd")
                            nc.vector.scalar_tensor_tensor(
                                out=sq, in0=xpc, scalar=1.0, in1=xpc,
                                op0=OP.mult, op1=OP.mult,
                                accum_out=ssq_p[:, j, q:q + 1],
                            )
                        else:
                            sq = sqp.tile([128, 512], F32, name="sqa", tag="sqa")
                            nc.scalar.activation(
                                sq, xpc, AF.Square,
                                accum_out=ssq_p[:, j, q:q + 1],
                            )
                    if ti == 0:
                        continue
                    for k in range(2):
                        c = 2 * cp + k
                        nc.tensor.matmul(
                            plg[:, 0:tw], lhsT=wn_s[:, c, :],
                            rhs=xt[:, c, 0:tw],
                            start=(c == 0), stop=(c == HC - 1),
                        )
                if ti == 0:
                    # preamble PE work runs behind tile-0 transposes
                    emit_wn_preamble()
                    for c in range(HC):
                        nc.tensor.matmul(
                            plg[:, 0:tw], lhsT=wn_s[:, c, :],
                            rhs=xt[:, c, 0:tw],
                            start=(c == 0), stop=(c == HC - 1),
                        )

                # reduce sumsq pieces, then rx = 1/max(sqrt(ssq), eps)
                ssq = scp.tile([128, NBLK], F32, name="ssq", tag="ssq")
                nc.vector.tensor_reduce(
                    out=ssq[:, 0:tb], in_=ssq_p[:, 0:tb, :], axis=AX.X,
                    op=OP.add,
                )
                ssqm = scp.tile([128, NBLK], F32, name="ssqm", tag="ssqm")
                nc.vector.tensor_scalar(
                    out=ssqm[:, 0:tb], in0=ssq[:, 0:tb], scalar1=EPS * EPS,
                    scalar2=None, op0=OP.max,
                )
                rx = emit_rsqrt(scp, ssqm, (128, NBLK), "rx", f_used=tb)

                # -- transpose back to [tok, E] blocks (full fp32) ---------
                lgT = epp.tile([64, TB], F32, name="lgT", tag="lgT")
                nc.scalar.copy(lgT[:, 0:tw], plg[:, 0:tw])
                ptb = psB.tile([128, NBLK, E], F32, name="ptb", tag="ptb")
                for j in range(tb):
                    nc.tensor.transpose(
                        ptb[:, j, :], lgT[:, j * 128:(j + 1) * 128],
                        ident_f[0:64, 0:64],
                    )

                # previous tile's epilogue drains while this tile streams
                if pending is not None:
                    emit_epilogue(*pending)
                pending = (b0, tb, ptb, rx)
            emit_epilogue(*pending)

    nc.compile()
    return nc


_NC_CACHE = {}


def _get_nc():
    if "nc" not in _NC_CACHE:
        _NC_CACHE["nc"] = build()
    return _NC_CACHE["nc"]


def _round_f32r(a):
    """Round fp32 to FP32R (11 explicit mantissa bits), nearest-even."""
    b = np.ascontiguousarray(a, dtype=np.float32).view(np.uint32)
    hi = b >> np.uint32(12)
    low = b & np.uint32(0xFFF)
    rnd = (low > np.uint32(0x800)) | (
        (low == np.uint32(0x800)) & ((hi & np.uint32(1)) == np.uint32(1))
    )
    out = (hi + rnd.astype(np.uint32)) << np.uint32(12)
    return out.view(np.float32)


def make_in_maps(x, sim_matrix, gates):
    x = _round_f32r(np.asarray(x, dtype=np.float32))
    sim = np.ascontiguousarray(np.asarray(sim_matrix, dtype=np.float32))
    g = np.ascontiguousarray(np.asarray(gates, dtype=np.float32)).reshape(1, E)
    return [
        {"x": x[c * NLOC:(c + 1) * NLOC], "sim": sim, "gates": g}
        for c in range(NCORES)
    ]


def kernel(x, sim_matrix, gates):
    nc = _get_nc()
    in_maps = make_in_maps(x, sim_matrix, gates)
    res = bass_utils.run_bass_kernel_spmd(nc, in_maps, core_ids=list(range(NCORES)))
    outs = []
    for name in ("mask", "probs", "logits"):
        outs.append(np.concatenate(
            [np.asarray(res.results[c][name], dtype=np.float32)
             for c in range(NCORES)], axis=0))
    return tuple(outs)
